# revision 11
# baseline (speedup 1.0000x reference)
"""Trainium2 Bass kernel for nn_CINLayer: out[b,d,o] = sum_{n,m} x[b,d,n]*y[b,d,m]*W[o,n*M+m].

Strategy (8-core data parallel over batch):
  Per sample s, out[o,s] = sum_k Wl[k,o] * Z[k,s] with Z[k,s] = x[s,n(k)]*y[s,m(k)].
  The contraction k (1600 products) is split into 13 chunks of 128 rows whose
  row->(n,m) mapping is chosen so each chunk's X-factor tile is a single
  DVE stream_shuffle of a host-staged interleaved layout Xil (per-quadrant
  lane-broadcast), and the Y-factor tiles are host-staged replicated layouts.
  Z chunks are built as one shuffle + one fp16 tensor_mul, then fed as the
  moving operand of fp16 matmuls accumulating out^T[o, s] in PSUM
  (o split 128+72, s tiles of 512).

  Chunk row mapping (r = 32j + r', j=quadrant):
    Part A (c<10):  (n, m) = (4c + j, r')          for r' < 32
    Part B (cb=c-10<3): r' = 8a + m''; (n, m) = (16cb + 4a + j, 32 + m'')
  Host layouts:
    Xil[32j + i]  = xT[4i + j]   (i<10, else 0)
    YrepA[p]      = yT[p % 32]
    YrepB[p]      = yT[32 + p % 8]
  Shuffle masks: A: mask[r'] = c ; B: mask[r'] = 4*cb + r'//8.
  W rows with n >= 40 (part B overhang) are zeroed on host.
"""

import numpy as np

BS, DIM, N, M, O = 2048, 32, 40, 40, 200
NCORES = 8
S_PER_CORE = BS * DIM // NCORES  # 8192
S_TILE = 512
N_STILES_FULL = S_PER_CORE // S_TILE  # 16
NCHUNKS = 13  # 10 part-A + 3 part-B
F16 = np.float16


def _chunk_row_to_nm(c: int, r: int):
    """Global chunk c (0..12), row r (0..127) -> (n, m) or None (zero pad)."""
    j, rp = divmod(r, 32)
    if c < 10:
        return 4 * c + j, rp
    cb = c - 10
    a, mpp = divmod(rp, 8)
    n = 16 * cb + 4 * a + j
    if n >= N:
        return None
    return n, 32 + mpp


def _shuffle_mask(c: int):
    if c < 10:
        return [c] * 32
    cb = c - 10
    return [4 * cb + (rp // 8) for rp in range(32)]


def _stage_w(W: np.ndarray) -> np.ndarray:
    """W [O, N*M] f32 -> wl [128, NCHUNKS, O] f16 (lhsT layout per chunk)."""
    Wr = W.reshape(O, N, M)
    wl = np.zeros((128, NCHUNKS, O), dtype=F16)
    for c in range(NCHUNKS):
        for r in range(128):
            nm = _chunk_row_to_nm(c, r)
            if nm is not None:
                wl[r, c, :] = Wr[:, nm[0], nm[1]].astype(F16)
    return wl


def _stage_core_inputs(x_flat: np.ndarray, y_flat: np.ndarray):
    """x_flat, y_flat [S_PER_CORE, 40] f32 -> xil, yrepa, yrepb [128, S] f16."""
    xT = np.ascontiguousarray(x_flat.T).astype(F16)  # [40, S]
    yT = np.ascontiguousarray(y_flat.T).astype(F16)  # [40, S]
    s = xT.shape[1]
    xil = np.zeros((128, s), dtype=F16)
    for p in range(128):
        j, i = divmod(p, 32)[0], p % 32
        if i < 10:
            xil[p] = xT[4 * i + j]
    yrepa = yT[np.arange(128) % 32]
    yrepb = yT[32 + (np.arange(128) % 8)]
    return xil, np.ascontiguousarray(yrepa), np.ascontiguousarray(yrepb)


def build_nc(n_stiles: int = N_STILES_FULL, debug: bool = False):
    """Build the per-core Bass/Tile module. Returns (nc, names dict)."""
    import concourse.bass as bass
    import concourse.tile as tile
    from concourse import bacc, mybir
    from concourse.tile_rust import add_dep_helper

    f16 = mybir.dt.float16
    f32 = mybir.dt.float32
    s_len = n_stiles * S_TILE

    nc = bacc.Bacc("TRN2", target_bir_lowering=False, debug=debug)

    xil_d = nc.dram_tensor("xil", [128, s_len], f16, kind="ExternalInput")
    ya_d = nc.dram_tensor("yrepa", [128, s_len], f16, kind="ExternalInput")
    yb_d = nc.dram_tensor("yrepb", [128, s_len], f16, kind="ExternalInput")
    wl_d = nc.dram_tensor("wl", [128, NCHUNKS, O], f16, kind="ExternalInput")
    out_d = nc.dram_tensor("outt", [O, s_len], f16, kind="ExternalOutput")

    with tile.TileContext(nc) as tc:
        with (
            tc.tile_pool(name="wpool", bufs=1) as wpool,
            tc.tile_pool(name="inp", bufs=3) as inp,
            tc.tile_pool(name="xe", bufs=6) as xep,
            tc.tile_pool(name="zp", bufs=6) as zp,
            tc.tile_pool(name="outp", bufs=3) as outp,
            tc.tile_pool(name="ps", bufs=2, space=bass.MemorySpace.PSUM) as psp,
        ):
            wl_sb = wpool.tile([128, NCHUNKS, O], f16)
            nc.sync.dma_start(wl_sb[:], wl_d[:])

            # xil lives in a persistent, manually double-buffered SBUF tensor:
            # its partition-strided replication reads are invisible to Tile's
            # dependency tracker, so RAW/WAR edges are added explicitly.
            xil_sb = nc.alloc_sbuf_tensor("xil_sb", [128, 2, S_TILE], f16)
            prev_reads = [[], []]

            for t in range(n_stiles):
                sl = bass.ts(t, S_TILE)
                slot = t % 2
                xslot = xil_sb.ap()[:, slot, :]
                w = nc.sync.dma_start(xslot, xil_d[:, sl])
                for r in prev_reads[slot]:
                    add_dep_helper(w.ins, r.ins, reason="xil slot WAR")
                reads = []
                ya_t = inp.tile([128, S_TILE], f16)
                nc.sync.dma_start(ya_t[:], ya_d[:, sl])
                yb_t = inp.tile([128, S_TILE], f16)
                nc.sync.dma_start(yb_t[:], yb_d[:, sl])

                psA = psp.tile([128, S_TILE], f32, tag="psA")
                psB = psp.tile([72, S_TILE], f32, tag="psB")

                for c in range(NCHUNKS):
                    z = zp.tile([128, S_TILE], f16)
                    xe = xep.tile([128, S_TILE], f16, tag="xe")
                    if c < 10:
                        # xe[32j+i, s] = xil[32j+c, s] (replication DMA)
                        src = (
                            xil_sb.ap()[c::32, slot, :]
                            .rearrange("a (b s) -> a b s", b=1)
                            .to_broadcast([4, 32, S_TILE])
                        )
                        r = nc.sync.dma_start(xe[:], src)
                        nc.vector.tensor_mul(z[:], ya_t[:], xe[:])
                    else:
                        r = nc.vector.stream_shuffle(
                            xe[:], xslot, _shuffle_mask(c)
                        )
                        nc.vector.tensor_mul(z[:], yb_t[:], xe[:])
                    add_dep_helper(r.ins, w.ins, reason="xil RAW")
                    reads.append(r)
                    first, last = c == 0, c == NCHUNKS - 1
                    nc.tensor.matmul(
                        psA[:], wl_sb[:, c, 0:128], z[:], start=first, stop=last
                    )
                    nc.tensor.matmul(
                        psB[:], wl_sb[:, c, 128:200], z[:], start=first, stop=last
                    )
                prev_reads[slot] = reads

                oA = outp.tile([128, S_TILE], f16, tag="oA")
                nc.scalar.copy(oA[:], psA[:])
                oB = outp.tile([72, S_TILE], f16, tag="oB")
                nc.scalar.copy(oB[:], psB[:])
                nc.sync.dma_start(out_d[0:128, sl], oA[:])
                nc.sync.dma_start(out_d[128:200, sl], oB[:])

    nc.compile()
    return nc


def kernel(x: np.ndarray, y: np.ndarray, W: np.ndarray) -> np.ndarray:
    from concourse.bass_utils import run_bass_kernel_spmd

    assert x.shape == (BS, DIM, N) and y.shape == (BS, DIM, M)
    assert W.shape == (O, N * M)

    wl = _stage_w(W)
    x_cores = x.reshape(NCORES, S_PER_CORE, N)
    y_cores = y.reshape(NCORES, S_PER_CORE, M)

    in_maps = []
    for i in range(NCORES):
        xil, yrepa, yrepb = _stage_core_inputs(x_cores[i], y_cores[i])
        in_maps.append({"xil": xil, "yrepa": yrepa, "yrepb": yrepb, "wl": wl})

    nc = build_nc()
    res = run_bass_kernel_spmd(nc, in_maps, core_ids=list(range(NCORES)))

    outs = []
    for i in range(NCORES):
        outt = res.results[i]["outt"]  # [O, S_PER_CORE] f16
        outs.append(outt.T.astype(np.float32))  # [S_PER_CORE, O]
    return np.concatenate(outs, axis=0).reshape(BS, DIM, O)


if __name__ == "__main__":
    xs = np.random.randn(BS, DIM, N).astype(np.float32)
    ys = np.random.randn(BS, DIM, M).astype(np.float32)
    Ws = (np.random.randn(O, N * M) * (1.0 / np.sqrt(N * M))).astype(np.float32)
    out = kernel(xs, ys, Ws)
    print(out.shape, out.dtype)


# revision 16
# speedup vs baseline: 2.6698x; 2.6698x over previous
"""Trainium2 Bass kernel for nn_CINLayer: out[b,d,o] = sum_{n,m} x[b,d,n]*y[b,d,m]*W[o,n*M+m].

Strategy (8-core data parallel over batch):
  Per sample s, out[o,s] = sum_k Wl[k,o] * Z[k,s] with Z[k,s] = x[s,n(k)]*y[s,m(k)].
  The contraction k (1600 products) is split into 13 chunks of 128 rows whose
  row->(n,m) mapping is chosen so each chunk's X-factor tile is a single
  DVE stream_shuffle of a host-staged interleaved layout Xil (per-quadrant
  lane-broadcast), and the Y-factor tiles are host-staged replicated layouts.
  Z chunks are built as one shuffle + one fp16 tensor_mul, then fed as the
  moving operand of fp16 matmuls accumulating out^T[o, s] in PSUM
  (o split 128+72, s tiles of 512).

  Chunk row mapping (r = 32j + r', j=quadrant):
    Part A (c<10):  (n, m) = (4c + j, r')          for r' < 32
    Part B (cb=c-10<3): r' = 8a + m''; (n, m) = (16cb + 4a + j, 32 + m'')
  Host layouts:
    Xil[32j + i]  = xT[4i + j]   (i<10, else 0)
    YrepA[p]      = yT[p % 32]
    YrepB[p]      = yT[32 + p % 8]
  Shuffle masks: A: mask[r'] = c ; B: mask[r'] = 4*cb + r'//8.
  W rows with n >= 40 (part B overhang) are zeroed on host.
"""

import numpy as np

BS, DIM, N, M, O = 2048, 32, 40, 40, 200
NCORES = 8
S_PER_CORE = BS * DIM // NCORES  # 8192
S_TILE = 512
N_STILES_FULL = S_PER_CORE // S_TILE  # 16
NCHUNKS = 13  # 10 part-A + 3 part-B
F16 = np.float16

# chunks whose Z-multiply runs on GPSIMD instead of DVE (load balance,
# spread through the chunk order so the PE is never tail-blocked)
GPSIMD_MULS = frozenset({2, 4, 6, 9, 11})


def _chunk_row_to_nm(c: int, r: int):
    """Global chunk c (0..12), row r (0..127) -> (n, m) or None (zero pad)."""
    j, rp = divmod(r, 32)
    if c < 10:
        return 4 * c + j, rp
    cb = c - 10
    a, mpp = divmod(rp, 8)
    n = 16 * cb + 4 * a + j
    if n >= N:
        return None
    return n, 32 + mpp


def _shuffle_mask(c: int):
    if c < 10:
        return [c] * 32
    cb = c - 10
    return [4 * cb + (rp // 8) for rp in range(32)]


def _stage_w(W: np.ndarray) -> np.ndarray:
    """W [O, N*M] f32 -> wl [128, NCHUNKS, O] f16 (lhsT layout per chunk)."""
    Wr = W.reshape(O, N, M)
    wl = np.zeros((128, NCHUNKS, O), dtype=F16)
    for c in range(NCHUNKS):
        for r in range(128):
            nm = _chunk_row_to_nm(c, r)
            if nm is not None:
                wl[r, c, :] = Wr[:, nm[0], nm[1]].astype(F16)
    return wl


def _stage_core_inputs(x_flat: np.ndarray, y_flat: np.ndarray):
    """x_flat, y_flat [S_PER_CORE, 40] f32 -> xil, yrepa, yrepb [128, S] f16."""
    xT = np.ascontiguousarray(x_flat.T).astype(F16)  # [40, S]
    yT = np.ascontiguousarray(y_flat.T).astype(F16)  # [40, S]
    s = xT.shape[1]
    xil = np.zeros((128, s), dtype=F16)
    for p in range(128):
        j, i = divmod(p, 32)[0], p % 32
        if i < 10:
            xil[p] = xT[4 * i + j]
    yrepa = yT[np.arange(128) % 32]
    yrepb = yT[32 + (np.arange(128) % 8)]
    return xil, np.ascontiguousarray(yrepa), np.ascontiguousarray(yrepb)


def build_nc(n_stiles: int = N_STILES_FULL, debug: bool = False):
    """Build the per-core Bass/Tile module. Returns (nc, names dict)."""
    import concourse.bass as bass
    import concourse.tile as tile
    from concourse import bacc, mybir
    from concourse.tile_rust import add_dep_helper

    f16 = mybir.dt.float16
    f32 = mybir.dt.float32
    s_len = n_stiles * S_TILE

    nc = bacc.Bacc("TRN2", target_bir_lowering=False, debug=debug)

    xil_d = nc.dram_tensor("xil", [128, s_len], f16, kind="ExternalInput")
    ya_d = nc.dram_tensor("yrepa", [128, s_len], f16, kind="ExternalInput")
    yb_d = nc.dram_tensor("yrepb", [128, s_len], f16, kind="ExternalInput")
    wl_d = nc.dram_tensor("wl", [128, NCHUNKS, O], f16, kind="ExternalInput")
    out_d = nc.dram_tensor("outt", [O, s_len], f16, kind="ExternalOutput")

    with tile.TileContext(nc) as tc:
        with (
            tc.tile_pool(name="wpool", bufs=1) as wpool,
            tc.tile_pool(name="inp", bufs=4) as inp,
            tc.tile_pool(name="xe", bufs=16) as xep,
            tc.tile_pool(name="zp", bufs=16) as zp,
            tc.tile_pool(name="outp", bufs=4) as outp,
            tc.tile_pool(name="ps", bufs=3, space=bass.MemorySpace.PSUM) as psp,
        ):
            wl_sb = wpool.tile([128, NCHUNKS, O], f16)
            nc.sync.dma_start(wl_sb[:], wl_d[:])

            for t in range(n_stiles):
                sl = bass.ts(t, S_TILE)
                xil_t = inp.tile([128, S_TILE], f16)
                nc.sync.dma_start(xil_t[:], xil_d[:, sl])
                ya_t = inp.tile([128, S_TILE], f16)
                nc.sync.dma_start(ya_t[:], ya_d[:, sl])
                yb_t = inp.tile([128, S_TILE], f16)
                nc.sync.dma_start(yb_t[:], yb_d[:, sl])

                psA = psp.tile([128, S_TILE], f32, tag="psA")
                psB = psp.tile([72, S_TILE], f32, tag="psB")

                for c in range(NCHUNKS):
                    xe = xep.tile([128, S_TILE], f16, tag="xe")
                    nc.vector.stream_shuffle(xe[:], xil_t[:], _shuffle_mask(c))
                    z = zp.tile([128, S_TILE], f16)
                    yt = ya_t if c < 10 else yb_t
                    eng = nc.gpsimd if c in GPSIMD_MULS else nc.vector
                    eng.tensor_mul(z[:], yt[:], xe[:])
                    first, last = c == 0, c == NCHUNKS - 1
                    nc.tensor.matmul(
                        psA[:], wl_sb[:, c, 0:128], z[:], start=first, stop=last
                    )
                    nc.tensor.matmul(
                        psB[:], wl_sb[:, c, 128:200], z[:], start=first, stop=last
                    )

                oA = outp.tile([128, S_TILE], f16, tag="oA")
                nc.scalar.copy(oA[:], psA[:])
                oB = outp.tile([72, S_TILE], f16, tag="oB")
                nc.scalar.copy(oB[:], psB[:])
                nc.sync.dma_start(out_d[0:128, sl], oA[:])
                nc.sync.dma_start(out_d[128:200, sl], oB[:])

    nc.compile()
    return nc


def kernel(x: np.ndarray, y: np.ndarray, W: np.ndarray) -> np.ndarray:
    from concourse.bass_utils import run_bass_kernel_spmd

    assert x.shape == (BS, DIM, N) and y.shape == (BS, DIM, M)
    assert W.shape == (O, N * M)

    wl = _stage_w(W)
    x_cores = x.reshape(NCORES, S_PER_CORE, N)
    y_cores = y.reshape(NCORES, S_PER_CORE, M)

    in_maps = []
    for i in range(NCORES):
        xil, yrepa, yrepb = _stage_core_inputs(x_cores[i], y_cores[i])
        in_maps.append({"xil": xil, "yrepa": yrepa, "yrepb": yrepb, "wl": wl})

    nc = build_nc()
    res = run_bass_kernel_spmd(nc, in_maps, core_ids=list(range(NCORES)))

    outs = []
    for i in range(NCORES):
        outt = res.results[i]["outt"]  # [O, S_PER_CORE] f16
        outs.append(outt.T.astype(np.float32))  # [S_PER_CORE, O]
    return np.concatenate(outs, axis=0).reshape(BS, DIM, O)


if __name__ == "__main__":
    xs = np.random.randn(BS, DIM, N).astype(np.float32)
    ys = np.random.randn(BS, DIM, M).astype(np.float32)
    Ws = (np.random.randn(O, N * M) * (1.0 / np.sqrt(N * M))).astype(np.float32)
    out = kernel(xs, ys, Ws)
    print(out.shape, out.dtype)
